# revision 2
# baseline (speedup 1.0000x reference)
"""DigitCaps routing-by-agreement kernel for 8 Trainium2 NeuronCores.

Math (faithful to the reference):
  u_hat[b,j,n,d] = sum_e x[b,n,e] W[j,n,d,e]
  iter1: c1 = 0.1 exactly (softmax of zeros)
         s1 = 0.1 * sum_n u_hat          -> GEMM, no u_hat materialization
         v1 = squash(s1)                 (GLOBAL scalar norm -> host reduce)
  iter2: t1[b,j,n] = sum_d v1 u_hat      -> per-j GEMM (G = W^T v1) + DVE
         c2 = softmax_j(t1) = E * R      (R folded into x: xR = x/R-sum)
         s2 = sum_n c2 u_hat             -> y = E*xR then per-j GEMM vs W
         v  = squash(s2)                 (global scalar -> host epilogue)

Sharding: pure data-parallel over batch (64 samples/core), W replicated.
Two NEFF launches; the global squash scalar between iterations is reduced
on the host (s1 is only [512,160] f32).

Engine balance in launch B (per TimelineSim cost model):
  PE   : G matmuls (e-diag trick) + swapped s2 matmuls (out free dim = 16)
  Act  : PSUM->SBUF G evictions + exp
  DVE  : x-multiplies (2x bf16 mode) + softmax sums + some reduce trees
  Pool : most of the e-reduce trees (gpsimd tensor_add)
Launch A runs the s1 GEMM in fp8 (x and W) - halves its DMA footprint;
numpy probe puts the induced final error at ~2.8e-3 (budget 2e-2).
"""

import numpy as np
import ml_dtypes
from contextlib import ExitStack

import concourse.bass as bass
import concourse.bacc as bacc
import concourse.tile as tile
import concourse.mybir as mybir
from concourse.bass_utils import run_bass_kernel_spmd

MCORES = 8
B, N, E, J, D = 512, 1152, 8, 10, 16
BC = B // MCORES            # 64 samples per core
NE = N * E                  # 9216
JD = J * D                  # 160
KC = NE // 128              # 72 k-chunks for the s1 GEMM
NCH = N // 128              # 9 n-chunks
EPS = 1e-7

F32 = mybir.dt.float32
BF16 = mybir.dt.bfloat16
FP8 = mybir.dt.float8e4
AX = mybir.AxisListType
ALU = mybir.AluOpType
ACTF = mybir.ActivationFunctionType

_BF = ml_dtypes.bfloat16
_F8 = ml_dtypes.float8_e4m3

_WA_COLS = J * NCH * 128            # 11520
_WS_COLS = J * E * NCH * D          # 11520
_XE_COLS = NCH * E * BC             # 4608
_VZ_COLS = E * J * BC               # 5120

HB = BC // 2          # 32: sub-batch half
EH = E * HB           # 256 cols per (ch, half) block
CW = NCH * EH         # 2304: full (ch,e,b') width per half

# units whose e-reduce tree runs on DVE (rest go to the idle Pool engine)
TREE_DVE = {(0, 0), (0, 4), (0, 8), (1, 2), (1, 6)}


def _bass():
    return bacc.Bacc("TRN2", target_bir_lowering=False, debug=False,
                     num_devices=MCORES)


def build_launch_a():
    """s1_raw[b, (j d)] = sum_{(n e)} xT[(n e), b] * Wk[(n e), (j d)].

    fp8 operands; f32 PSUM accumulation. Four x/W piece pairs sized so the
    first matmuls start early and later pieces stream under compute.
    """
    nc = _bass()
    xT2 = nc.dram_tensor("xT2", [128, KC * BC], FP8, kind="ExternalInput").ap()
    Wk2 = nc.dram_tensor("Wk2", [128, KC * JD], FP8, kind="ExternalInput").ap()
    s1 = nc.dram_tensor("s1", [BC, JD], F32, kind="ExternalOutput").ap()

    pieces = [0, 12, 30, 51, 72]

    with tile.TileContext(nc) as tc, ExitStack() as ctx:
        io = ctx.enter_context(tc.tile_pool(name="io", bufs=1))
        ps = ctx.enter_context(tc.tile_pool(name="ps", bufs=1, space="PSUM"))
        sb = ctx.enter_context(tc.tile_pool(name="sb", bufs=1))

        xT_sb = io.tile([128, KC * BC], FP8)
        Wk_sb = io.tile([128, KC * JD], FP8)
        for lo, hi in zip(pieces, pieces[1:]):
            nc.sync.dma_start(xT_sb[:, lo * BC:hi * BC],
                              xT2[:, lo * BC:hi * BC])
            nc.scalar.dma_start(Wk_sb[:, lo * JD:hi * JD],
                                Wk2[:, lo * JD:hi * JD])

        acc = ps.tile([BC, JD], F32)
        for k in range(KC):
            nc.tensor.matmul(
                acc[:],
                lhsT=xT_sb[:, k * BC:(k + 1) * BC],
                rhs=Wk_sb[:, k * JD:(k + 1) * JD],
                start=(k == 0), stop=(k == KC - 1),
            )
        out_sb = sb.tile([BC, JD], F32)
        nc.scalar.copy(out_sb[:], acc[:])
        nc.sync.dma_start(s1, out_sb[:])
    nc.compile()
    return nc


def build_launch_b():
    """Routing iteration 2, fully on chip except the squash scalars.

    Inputs (host layout):
      WAd [128, 11520] fp8 : block (j,ch) = [128,128] lhsT, rows e*16+d =
                             W[j, ch*128+n', d, e]
      WSd [128, 11520] bf16: slice (j,e,ch) = [128, D] rhs, row n' =
                             W[j, ch*128+n', d, e]
      xv  [128, 9728] bf16 = [ v1z | xE_h0 | xE_h1 ]
        v1z col j*512 + e*64 + b; rows e*16..+16 = v1T[d,b] for j
        xE_h[n=ch*128+p, (ch,e,b')] = x[h*32+b', n, e]
    Output: s2_raw [BC, JD] f32  (s2_raw[b, j*16+d] = s2[b, j, d])
    """
    nc = _bass()
    WAd = nc.dram_tensor("WAd", [128, _WA_COLS], FP8,
                         kind="ExternalInput").ap()
    WSd = nc.dram_tensor("WSd", [128, _WS_COLS], BF16,
                         kind="ExternalInput").ap()
    xv = nc.dram_tensor("xv", [128, _XE_COLS + _VZ_COLS], BF16,
                        kind="ExternalInput").ap()
    s2 = nc.dram_tensor("s2", [BC, JD], F32, kind="ExternalOutput").ap()

    with tile.TileContext(nc) as tc, ExitStack() as ctx:
        io = ctx.enter_context(tc.tile_pool(name="io", bufs=1))
        psA = ctx.enter_context(tc.tile_pool(name="psA", bufs=3, space="PSUM"))
        psS = ctx.enter_context(tc.tile_pool(name="psS", bufs=2, space="PSUM"))
        stage = ctx.enter_context(tc.tile_pool(name="stage", bufs=3))
        soft = ctx.enter_context(tc.tile_pool(name="soft", bufs=2))
        big = ctx.enter_context(tc.tile_pool(name="big", bufs=1))

        WA_sb = io.tile([128, _WA_COLS], FP8)
        WS_sb = io.tile([128, _WS_COLS], BF16)
        xv_sb = io.tile([128, _XE_COLS + _VZ_COLS], BF16)

        VJ = E * BC                       # one j-block of v1z (512 cols)
        qa = _WA_COLS // 4
        # ordered by first use: v1z j0/j1, WA q1, xE h0, WA q2, v1z rest,
        # WA q3+q4, xE h1, WS halves last (b3-only)
        def dmas():
            yield nc.scalar, xv_sb, xv, 0, 2 * VJ
            yield nc.sync, WA_sb, WAd, 0, qa
            yield nc.scalar, xv_sb, xv, _VZ_COLS, _VZ_COLS + CW
            yield nc.sync, WA_sb, WAd, qa, 2 * qa
            yield nc.scalar, xv_sb, xv, 2 * VJ, _VZ_COLS
            yield nc.sync, WA_sb, WAd, 2 * qa, 4 * qa
            yield nc.scalar, xv_sb, xv, _VZ_COLS + CW, _VZ_COLS + 2 * CW
            yield nc.sync, WS_sb, WSd, 0, _WS_COLS // 2
            yield nc.scalar, WS_sb, WSd, _WS_COLS // 2, _WS_COLS
        for eng, dst, srcT, lo, hi in dmas():
            eng.dma_start(dst[:, lo:hi], srcT[:, lo:hi])

        def WA_blk(j, ch):
            o = (j * NCH + ch) * 128
            return WA_sb[:, o:o + 128]

        def WS_slc(j, e, ch):
            o = ((j * E + e) * NCH + ch) * D
            return WS_sb[:, o:o + D]

        # v1z region viewed [p, j, e, b]
        v1_v = xv_sb[:, 0:_VZ_COLS] \
            .rearrange("p (jj e b) -> p jj e b", e=E, jj=J)

        def xE_h(h):
            o = _VZ_COLS + h * CW
            return xv_sb[:, o:o + CW]

        def xE_h4(h):
            return xE_h(h).rearrange("p (c e b) -> p c e b", c=NCH, e=E)

        # per-half persistent tiles
        t1_h = [big.tile([128, NCH * J * HB], BF16, tag=f"t1{h}",
                         name=f"t1_h{h}") for h in range(2)]
        Ex_h = [big.tile([128, NCH * J * HB], BF16, tag=f"Ex{h}",
                         name=f"Ex_h{h}") for h in range(2)]
        xR_h = [big.tile([128, CW], BF16, tag=f"xR{h}",
                         name=f"xR_h{h}") for h in range(2)]
        s2_sb = big.tile([BC, JD], F32, tag="s2o", name="s2_sb")

        def t1_v(h):
            return t1_h[h][:].rearrange("p (c j b) -> p c j b", c=NCH, j=J)

        def Ex_v(h):
            return Ex_h[h][:].rearrange("p (c j b) -> p c j b", c=NCH, j=J)

        def b1_unit(h, j):
            tree = nc.vector if (h, j) in TREE_DVE else nc.gpsimd
            rhs = v1_v[:, j][:, :, h * HB:(h + 1) * HB]        # [p,8,32]
            Ps = stage.tile([128, CW], BF16, tag="Ps", name=f"Ps_{h}_{j}")
            for ch2 in range(3):
                lo, hi = ch2 * 3, ch2 * 3 + 3
                acc = psA.tile([128, (hi - lo) * EH], F32, tag="pA",
                               name=f"acc_{h}_{j}_{ch2}")
                for ch in range(lo, hi):
                    nc.tensor.matmul(
                        acc[:, (ch - lo) * EH:(ch - lo + 1) * EH]
                            .rearrange("p (e b) -> p e b", e=E),
                        lhsT=WA_blk(j, ch), rhs=rhs,
                        start=True, stop=True,
                    )
                nc.scalar.copy(Ps[:, lo * EH:hi * EH], acc[:])
            # Pm = Ps * x (layouts match: both (ch,e,b)), then e-tree
            Pm = stage.tile([128, CW], BF16, tag="Pm", name=f"Pm_{h}_{j}")
            nc.vector.tensor_mul(Pm[:], Ps[:], xE_h(h))
            Pm4 = Pm[:].rearrange("p (c e b) -> p c e b", c=NCH, e=E)
            T1 = stage.tile([128, NCH * 4 * HB], BF16, tag="T1",
                            name=f"T1_{h}_{j}")
            T1v = T1[:].rearrange("p (c e b) -> p c e b", c=NCH, e=4)
            tree.tensor_add(T1v, Pm4[:, :, 0:4], Pm4[:, :, 4:8])
            T2 = stage.tile([128, NCH * 2 * HB], BF16, tag="T2",
                            name=f"T2_{h}_{j}")
            T2v = T2[:].rearrange("p (c e b) -> p c e b", c=NCH, e=2)
            tree.tensor_add(T2v, T1v[:, :, 0:2], T1v[:, :, 2:4])
            tree.tensor_add(
                t1_v(h)[:, :, j:j + 1],
                T2v[:, :, 0:1], T2v[:, :, 1:2],
            )

        def b2_unit(h):
            # E = exp(t1); SE = sum_j E; xR = x / SE (normalizer folded
            # into x so c2 never materializes: y_j = E_j * xR)
            nc.scalar.activation(Ex_h[h][:], t1_h[h][:], ACTF.Exp)
            Ev = Ex_v(h)
            S5 = soft.tile([128, NCH * 5 * HB], BF16, tag="S5",
                           name=f"S5_{h}")
            S5v = S5[:].rearrange("p (c j b) -> p c j b", c=NCH, j=5)
            nc.vector.tensor_add(S5v, Ev[:, :, 0:5], Ev[:, :, 5:10])
            S2 = soft.tile([128, NCH * 2 * HB], BF16, tag="S2",
                           name=f"S2_{h}")
            S2v = S2[:].rearrange("p (c j b) -> p c j b", c=NCH, j=2)
            nc.vector.tensor_add(S2v, S5v[:, :, 0:2], S5v[:, :, 2:4])
            S1 = soft.tile([128, NCH * HB], F32, tag="S1", name=f"S1_{h}")
            S1v = S1[:].rearrange("p (c b) -> p c b", c=NCH)
            nc.vector.tensor_add(S1v.unsqueeze(2),
                                 S2v[:, :, 0:1], S2v[:, :, 1:2])
            SE = soft.tile([128, NCH * HB], F32, tag="SE", name=f"SE_{h}")
            SEv = SE[:].rearrange("p (c b) -> p c b", c=NCH)
            nc.vector.tensor_add(SEv.unsqueeze(2), S1v.unsqueeze(2),
                                 S5v[:, :, 4:5])
            Re = soft.tile([128, NCH * HB], F32, tag="Re", name=f"Re_{h}")
            nc.vector.reciprocal(Re[:], SE[:])
            Rb = soft.tile([128, NCH * HB], BF16, tag="Rb", name=f"Rb_{h}")
            nc.vector.tensor_copy(Rb[:], Re[:])
            nc.vector.tensor_mul(
                xR_h[h][:].rearrange("p (c e b) -> p c e b", c=NCH, e=E),
                xE_h4(h),
                Rb[:].rearrange("p (c b) -> p c b", c=NCH)
                    .unsqueeze(2).broadcast_to([128, NCH, E, HB]),
            )

        def b3_unit(h, j):
            y_j = stage.tile([128, CW], BF16, tag="yj", name=f"yj_{h}_{j}")
            nc.vector.tensor_mul(
                y_j[:].rearrange("p (c e b) -> p c e b", c=NCH, e=E),
                xR_h[h][:].rearrange("p (c e b) -> p c e b", c=NCH, e=E),
                Ex_v(h)[:, :, j]
                    .unsqueeze(2).broadcast_to([128, NCH, E, HB]),
            )
            # s2^T chunk: out[b', d] = sum_n y[n, b'] W[n, d]; rhs free
            # dim 16 halves PE time vs the [16, 32] orientation
            acc2 = psS.tile([HB, D], F32, tag="pS", name=f"acc2_{h}_{j}")
            for ch in range(NCH):
                for e in range(E):
                    nc.tensor.matmul(
                        acc2[:],
                        lhsT=y_j[:, (ch * E + e) * HB:(ch * E + e + 1) * HB],
                        rhs=WS_slc(j, e, ch),
                        start=(ch == 0 and e == 0),
                        stop=(ch == NCH - 1 and e == E - 1),
                    )
            nc.scalar.copy(
                s2_sb[h * HB:(h + 1) * HB, j * D:(j + 1) * D], acc2[:])

        # half 0 logits+softmax; then interleave its B3 with half 1's B1 so
        # PE/Act work of one hides under the DVE/Pool work of the other.
        for j in range(J):
            b1_unit(0, j)
        b2_unit(0)
        for j in range(J):
            b3_unit(0, j)
            b1_unit(1, j)
        b2_unit(1)
        for j in range(J):
            b3_unit(1, j)
        nc.sync.dma_start(s2, s2_sb[:])
    nc.compile()
    return nc


_cache = {}


def _get_programs():
    if "a" not in _cache:
        _cache["a"] = build_launch_a()
        _cache["b"] = build_launch_b()
    return _cache["a"], _cache["b"]


def _prep_host(x, W):
    xf = np.ascontiguousarray(x, dtype=np.float32)
    Wf = np.ascontiguousarray(W, dtype=np.float32)

    # Launch A weights: Wk[(n e), (j d)] = W[j,n,d,e], chunked to [128, KC*JD]
    Wk = Wf.transpose(1, 3, 0, 2).reshape(NE, JD)
    Wk2 = np.ascontiguousarray(
        Wk.reshape(KC, 128, JD).transpose(1, 0, 2).reshape(128, KC * JD)
    ).astype(_F8)

    # WA block (j,ch): rows e*16+d, cols n' -> W[j, ch*128+n', d, e]
    WAt = Wf.transpose(3, 2, 0, 1).reshape(E * D, J, NCH, 128)
    WA = np.ascontiguousarray(
        WAt.reshape(E * D, J * NCH * 128)).astype(_F8)

    # WS slice (j,e,ch): [128, D] rows n' -> W[j, ch*128+n', d, e]
    WSt = Wf.transpose(1, 0, 3, 2).reshape(NCH, 128, J, E, D)
    WS = np.ascontiguousarray(
        WSt.transpose(1, 2, 3, 0, 4).reshape(128, J * E * NCH * D)
    ).astype(_BF)

    # Per-core x layouts
    xs = xf.reshape(MCORES, BC, N, E)
    xT2s, xEs = [], []
    for c in range(MCORES):
        xT = xs[c].transpose(1, 2, 0).reshape(NE, BC)           # [(n e), b]
        xT2s.append(np.ascontiguousarray(
            xT.reshape(KC, 128, BC).transpose(1, 0, 2).reshape(128, KC * BC)
        ).astype(_F8))
        xE = xs[c].transpose(1, 2, 0).reshape(N, E * BC)        # [n, (e b)]
        xEs.append(np.ascontiguousarray(
            xE.reshape(NCH, 128, E * BC).transpose(1, 0, 2)
              .reshape(128, NCH * E * BC)))
    return Wk2, WA, WS, xT2s, xEs


def kernel(x, W):
    nc_a, nc_b = _get_programs()
    Wk2, WA, WS, xT2s, xEs = _prep_host(x, W)
    core_ids = list(range(MCORES))

    in_a = [{"xT2": xT2s[c], "Wk2": Wk2} for c in core_ids]
    res_a = run_bass_kernel_spmd(nc_a, in_a, core_ids).results
    s1_raw = np.stack([res_a[c]["s1"] for c in core_ids])       # [M, BC, JD]

    s1 = 0.1 * s1_raw.reshape(B, J, D).astype(np.float32)
    sq1 = float(np.sum(s1.astype(np.float64) ** 2))
    g1 = sq1 / (1.0 + sq1) / np.sqrt(sq1 + EPS)
    v1 = (g1 * s1).astype(np.float32)                           # [B, J, D]

    # v1z per core (j-major): col j*512+e*64+b; rows e*16+d = v1T[d,b]
    v1T = v1.reshape(MCORES, BC, J, D)
    in_b = []
    for c in range(MCORES):
        vt = v1T[c].transpose(2, 1, 0)                          # [d, j, b]
        v4 = np.zeros((E, D, J, E, BC), np.float32)
        for e in range(E):
            v4[e, :, :, e, :] = vt
        v1z = v4.reshape(128, J * E * BC)
        xE9 = xEs[c].reshape(128, NCH, E, BC)
        xh0 = np.ascontiguousarray(xE9[:, :, :, 0:BC // 2]).reshape(128, -1)
        xh1 = np.ascontiguousarray(xE9[:, :, :, BC // 2:]).reshape(128, -1)
        xvc = np.concatenate([v1z, xh0, xh1], axis=1).astype(_BF)
        in_b.append({"WAd": WA, "WSd": WS, "xv": xvc})
    res_b = run_bass_kernel_spmd(nc_b, in_b, core_ids).results
    s2_raw = np.stack([res_b[c]["s2"] for c in core_ids])       # [M, BC, JD]

    s2 = s2_raw.reshape(B, J, D).astype(np.float32)
    sq2 = float(np.sum(s2.astype(np.float64) ** 2))
    g2 = sq2 / (1.0 + sq2) / np.sqrt(sq2 + EPS)
    return (g2 * s2).astype(np.float32)
